# revision 32
# baseline (speedup 1.0000x reference)
"""AttnReadout kernel for Trainium2, 8 NeuronCores, data-parallel over batch.

Math (per batch b, head i):
  c[i,e]    = bu[i,e] + sum_d Wv[i,e,d] * x[b, i, last_nodes[b,i], d]
  z[t,e]    = sum_d x[b,t,d] * Wu[i,e,d]          (t over O*N = 8192 tokens)
  s[t,e]    = sigmoid(z[t,e] + c[i,e])
  score[t]  = sum_e We[i,e] * s[t,e]
  alpha     = softmax(score)        (scores bounded by |We|_1, so exp without
                                     max-subtraction is safe; softmax is
                                     shift-invariant so results match)
  out[b,i]  = sum_t alpha[t] * x[b,t,:]

Trick: sigmoid(v) = (1 + tanh(v/2))/2 and the We-dot is linear, so
  score = sum_e (We_e/2)*tanh((z_e + c_e)/2) + sum_e We_e/2
Using tanh keeps every ACT function (tanh, exp, identity) in the single
`exp_and_others` table set -> no ACT table reloads. The /2 factors are
folded into the uploaded weights (exact in bf16), the +sum(We)/2 into the
exp bias.

v2 pipeline (per core, 4 samples), driven by trace analysis of v1:
the kernel is a 2-stream race: the PE LDWEIGHTS stream (~1 col/cycle @
1.2 GHz; score-matmul stationaries dominate it) and the ACT stream
(1 elem/lane/cyc @ 1.2 GHz + ~352 cyc/call overhead). v1 lost ~35 us to
startup (12.5 us to first tanh), sample-boundary PE-queue clogs
(32-matmul wsum blocks), and tanh call-count overhead (64 calls/core).

v2:
  - z regions double-buffer across two 3-bank PSUM pools (1536 f32 each);
    6 tanh calls per head (48/core vs 64), A/B alternation lines up across
    heads so no pool-reuse stall.
  - per-sample scores/u/Z tile occupies one PSUM bank (scp, bufs=2); Z
    goes to the bank's padding bytes so nothing overlaps.
  - all small weights ship as one blob DMA; xt slices lead the DMA ring;
    warmup matmuls + the ch bias chain fill the HAM-cold window, so the
    first tanh fires at ~3.5 us instead of 12.5.
  - wsum (weighted-sum) matmuls spread 8-per-slot across the next
    sample's slots so the in-order PE queue never starves ACT.
"""

import numpy as np
import ml_dtypes

import concourse.bacc as bacc
import concourse.tile as tile
from concourse import mybir
from concourse.bass_utils import run_bass_kernel_spmd

BF = ml_dtypes.bfloat16
B, O, N, D = 32, 2, 4096, 128
NCORES = 8
BPC = B // NCORES          # samples per core
T = O * N                  # tokens per head (attention span is O*N)
NC64 = T // 128            # 64 token chunks of 128
WSB = 4                    # wsum chunks per emitted block

# z regions per head, balanced so no slot leaves the PE idle long enough
# to trip the HAM re-throttle; the first head starts with small regions so
# the ACT stream lights up as early as possible. Pools are picked by a
# global alternation counter. Pools hold 1536 f32 = 3 PSUM banks.
SIZES = [1024, 1536, 1536, 1536, 1536, 1024]
SIZES_FIRST = [512, 1024, 1536, 1536, 1536, 1536, 512]
ZCAP = 1536

# blob column layout (bf16): wuT [0:256], wvT [256:512], we2 [512:514],
# xlT [514:522], ones [522:523]
WU0, WV0, WE0, XL0, ON0, BLOBW = 0, 256, 512, 514, 522, 523


def _build_program():
    nc = bacc.Bacc("TRN2", target_bir_lowering=False)
    dt = mybir.dt
    f32, bf16 = dt.float32, dt.bfloat16

    xt_d = nc.dram_tensor("xt", [BPC, D, T], bf16, kind="ExternalInput")
    xn_d = nc.dram_tensor("xn", [BPC, 2, D, 32 * (D + 1)], bf16, kind="ExternalInput")
    wb_d = nc.dram_tensor("wb", [D, BLOBW], bf16, kind="ExternalInput")
    fb_d = nc.dram_tensor("fb", [D, 4], f32, kind="ExternalInput")  # bu|cw
    out_d = nc.dram_tensor("out", [BPC, O, D], f32, kind="ExternalOutput")

    Tanh = mybir.ActivationFunctionType.Tanh
    Exp = mybir.ActivationFunctionType.Exp
    Ident = mybir.ActivationFunctionType.Identity

    with tile.TileContext(nc) as tc:
        from contextlib import ExitStack

        with ExitStack() as ctx:
            singles = ctx.enter_context(tc.tile_pool(name="singles", bufs=1))
            xtp = ctx.enter_context(tc.tile_pool(name="xtp", bufs=3))
            xnp = ctx.enter_context(tc.tile_pool(name="xnp", bufs=3))
            za = ctx.enter_context(tc.tile_pool(name="za", bufs=1, space="PSUM"))
            zb = ctx.enter_context(tc.tile_pool(name="zb", bufs=1, space="PSUM"))
            scp = ctx.enter_context(tc.tile_pool(name="scp", bufs=2, space="PSUM"))
            sp = ctx.enter_context(tc.tile_pool(name="sp", bufs=4))
            smalls = ctx.enter_context(tc.tile_pool(name="smalls", bufs=2))

            wb_sb = singles.tile([D, BLOBW], bf16)
            nc.sync.dma_start(out=wb_sb, in_=wb_d[:])
            fb_sb = singles.tile([D, 4], f32)
            nc.sync.dma_start(out=fb_sb, in_=fb_d[:])

            wu = lambda i: wb_sb[:, WU0 + i * D : WU0 + (i + 1) * D]
            wv = lambda i: wb_sb[:, WV0 + i * D : WV0 + (i + 1) * D]
            we = lambda i: wb_sb[:, WE0 + i : WE0 + i + 1]
            ones_col = wb_sb[:, ON0 : ON0 + 1]

            samples = {}

            def preload(b):
                xt_sb = xtp.tile([D, T], bf16, tag="xt")
                bounds = (
                    [0, 512, 1536, 3072, 4608, 6144, 7680, T]
                    if b == 0
                    else [0, T // 2, T]
                )
                for lo, hi in zip(bounds[:-1], bounds[1:]):
                    nc.sync.dma_start(out=xt_sb[:, lo:hi], in_=xt_d[b, :, lo:hi])
                xn_sb = xnp.tile([D, NC64, D + 1], bf16, tag="xn")
                for g in range(2):
                    nc.sync.dma_start(
                        out=xn_sb[:, g * 32 : (g + 1) * 32, :],
                        in_=xn_d[b, g].rearrange("p (c d) -> p c d", c=32),
                    )
                # per-sample PSUM bank: scores [0:128] (viewed [i, c]),
                # u [128:257] on partitions 0-1 -- the xn ones column makes
                # u[:, 128] the softmax normalizer Z, for free.
                scu = scp.tile([D, 512], f32, tag="scu")
                scores = scu[:, 0:128].rearrange("p (i c) -> p i c", i=O)
                if b == BPC - 1:
                    # the last sample's weighted sum overlaps its own scoring;
                    # its u lives in the previous sample's (finished) bank so
                    # this sample's wedot start=True bank clears cannot wipe
                    # the accumulation bits mid-flight.
                    u_ap = samples[b - 1][5][0:2, 128:257]
                else:
                    u_ap = scu[0:2, 128:257]
                alpha_sb = smalls.tile([D, NC64, O], bf16, tag="alpha")
                samples[b] = (xt_sb, xn_sb, scores, u_ap, alpha_sb, scu)

            def emit_wedot(slot, b, i, col0, t_ap, size):
                scores = samples[b][2]
                for sub in range(size // D):
                    col = col0 + sub
                    nc.tensor.matmul(
                        scores[:, i, col : col + 1],
                        t_ap[:, sub * D : (sub + 1) * D],
                        we(i),
                        start=True,
                        stop=True,
                    )
                subs = size // D
                if b == BPC - 1 and i == O - 1:
                    # last sample, head 1: exp is chunk-local (raw exp, no
                    # max-subtraction), so score regions convert to alpha as
                    # they are produced and the weighted sum overlaps its own
                    # sample's scoring instead of draining serially at the end
                    exp_queue.append((slot, b, i, col0, subs))
                elif (col0 + subs) == NC64:
                    exp_queue.append((slot, b, i, 0, NC64))

            exp_queue = []    # (slot_pushed, b, i, col0, ncols)
            blocks_queued = [0] * BPC

            def emit_exp(b, i, col0, ncols):
                # runs >=1 slot after its wedot so the ACT FIFO never stalls
                # on the PE finishing the score columns
                scores, alpha_sb = samples[b][2], samples[b][4]
                nc.scalar.activation(
                    out=alpha_sb[:, col0 : col0 + ncols, i],
                    in_=scores[:, i, col0 : col0 + ncols],
                    func=Exp,
                    bias=fb_sb[:, 2 + i : 3 + i],
                )
                if i == O - 1:
                    ready = (col0 + ncols) // WSB
                    while blocks_queued[b] < ready:
                        deferred_wsum.append((b, blocks_queued[b]))
                        blocks_queued[b] += 1

            deferred_wsum = []  # (b, block_idx) queue

            def emit_wsum_block():
                b, blk = deferred_wsum.pop(0)
                _, xn_sb, _, u_ap, alpha_sb, _ = samples[b]
                for k in range(WSB):
                    c = blk * WSB + k
                    nc.tensor.matmul(
                        u_ap,
                        alpha_sb[:, c, :],
                        xn_sb[:, c, :],
                        start=(c == 0),
                        stop=(c == NC64 - 1),
                    )
                if blk == NC64 // WSB - 1:
                    zinv_sb = smalls.tile([O, 1], f32, tag="zinv")
                    nc.vector.reciprocal(out=zinv_sb, in_=u_ap[:, 128:129])
                    o_sb = smalls.tile([O, D], f32, tag="osb")
                    nc.vector.tensor_scalar_mul(o_sb, u_ap[:, 0:128], zinv_sb)
                    nc.sync.dma_start(out=out_d[b], in_=o_sb)

            preload(0)

            # startup: warmup matmuls heat the PE HAM clock gate while the
            # first xt slices stream in. The ch bias chain (c_ps in sample
            # 0's scu spare columns) interleaves so proj r0 is not gated on
            # a z-pool tile.
            zwarm = za.tile([D, ZCAP], f32, tag="z")
            scu0 = samples[0][5]
            for w in range(4):
                nc.tensor.matmul(zwarm[:, 0:D], wu(0), wv(1), start=True, stop=True)
            # per-(sample, head) tanh bias ch[e, j] = (xv + bu)/2, j = i*BPC + b
            # (wv and bu are uploaded pre-halved)
            for i in range(O):
                nc.tensor.matmul(
                    scu0[:, 264 + i * BPC : 264 + (i + 1) * BPC],
                    wv(i),
                    wb_sb[:, XL0 + i * BPC : XL0 + (i + 1) * BPC],
                    start=True,
                    stop=True,
                )
            ch_sb = singles.tile([D, O * BPC], f32)
            for i in range(O):
                nc.scalar.activation(
                    out=ch_sb[:, i * BPC : (i + 1) * BPC],
                    in_=scu0[:, 264 + i * BPC : 264 + (i + 1) * BPC],
                    func=Ident,
                    bias=fb_sb[:, i : i + 1],
                )
            for w in range(2):
                nc.tensor.matmul(zwarm[:, 0:D], wu(0), wv(1), start=True, stop=True)

            pending = None
            grc = 0  # global region counter: picks the A/B pool alternately
            slot = 0
            for b in range(BPC):
                for i in range(O):
                    off = 0
                    sizes = SIZES_FIRST if (b, i) == (0, 0) else SIZES
                    for r, size in enumerate(sizes):
                        if i == 0 and r == 0 and b + 1 < BPC:
                            preload(b + 1)
                        z = (za if grc % 2 == 0 else zb).tile([D, ZCAP], f32, tag="z")
                        grc += 1
                        xt_sb = samples[b][0]
                        for p0 in range(0, size, 512):
                            p1 = min(p0 + 512, size)
                            nc.tensor.matmul(
                                z[:, p0:p1],
                                wu(i),
                                xt_sb[:, off + p0 : off + p1],
                                start=True,
                                stop=True,
                            )
                        if pending is not None:
                            emit_wedot(slot, *pending)
                        if deferred_wsum:
                            emit_wsum_block()
                        t_sb = sp.tile([D, ZCAP], bf16, tag="t")
                        j = i * BPC + b
                        nc.scalar.activation(
                            out=t_sb[:, :size],
                            in_=z[:, :size],
                            func=Tanh,
                            bias=ch_sb[:, j : j + 1],
                        )
                        while exp_queue and exp_queue[0][0] < slot - 2:
                            _, eb, ei, ecol0, encols = exp_queue.pop(0)
                            emit_exp(eb, ei, ecol0, encols)
                        pending = (b, i, off // D, t_sb[:, :size], size)
                        off += size
                        slot += 1
            emit_wedot(slot, *pending)
            while exp_queue:
                _, eb, ei, ecol0, encols = exp_queue.pop(0)
                emit_exp(eb, ei, ecol0, encols)
            while deferred_wsum:
                emit_wsum_block()

    nc.compile()
    return nc


def _prep_core_inputs(x, Wu, bu, Wv, We, last_nodes):
    """Host-side input marshalling: dtype cast + layout (weights pre-halved
    for the tanh formulation). Returns per-core input maps."""
    x = np.ascontiguousarray(x, dtype=np.float32)
    ln = np.asarray(last_nodes).astype(np.int64)
    xb = x.reshape(B, T, D)
    xbf = xb.astype(BF)                                  # [B, T, D] bf16
    xt = np.ascontiguousarray(xbf.transpose(0, 2, 1))    # [B, D, T]
    # natural-chunked layout with a trailing ones column per chunk (so the
    # weighted sum also produces the softmax normalizer Z):
    # xn[b, g, p, cc*(D+1) + d] = xb[b, (g*32 + cc)*128 + p, d]; d=D slot = 1
    xn5 = xbf.reshape(B, 2, 32, D, D).transpose(0, 1, 3, 2, 4)   # [b,g,p,cc,d]
    xn = np.empty((B, 2, D, 32, D + 1), BF)
    xn[..., :D] = xn5
    xn[..., D] = BF(1.0)
    xn = np.ascontiguousarray(xn.reshape(B, 2, D, 32 * (D + 1)))
    # x_last gather, transposed: xlT[d, j], j = i*BPC + b_local
    xl = xb[np.arange(B)[:, None], ln + np.arange(O)[None, :] * N]   # [B, O, D] f32
    # wuT[d, i, e] = Wu[i, e, d] / 2  (tanh halving, exact in bf16)
    wuT = (Wu * 0.5).transpose(2, 0, 1).reshape(D, O * D).astype(BF)
    wvT = (Wv * 0.5).transpose(2, 0, 1).reshape(D, O * D).astype(BF)
    we2 = (We * 0.5).astype(BF).T                        # [e, i] = We[i, e]/2
    bu2 = np.ascontiguousarray((bu * 0.5).astype(np.float32).T)  # [e, i]
    # exp bias: cw[i] = sum_e We[i, e]/2, replicated on all partitions
    cw = np.float32(0.5) * We.astype(np.float32).sum(axis=1)     # [O]
    cw2 = np.broadcast_to(cw[None, :], (D, O)).astype(np.float32)
    fb = np.ascontiguousarray(np.concatenate([bu2, cw2], axis=1))  # [D, 4]
    ones = np.ones((D, 1), BF)

    maps = []
    for core in range(NCORES):
        sl = slice(core * BPC, (core + 1) * BPC)
        xlc = xl[sl]                                     # [BPC, O, D]
        xlT = xlc.transpose(2, 1, 0).reshape(D, O * BPC).astype(BF)
        wb = np.ascontiguousarray(
            np.concatenate([wuT, wvT, we2, xlT, ones], axis=1)
        )                                                # [D, 523] bf16
        maps.append({"xt": xt[sl], "xn": xn[sl], "wb": wb, "fb": fb})
    return maps


_CACHE = {}
TRACE = False


def kernel(**inputs):
    x = np.asarray(inputs["x"])
    Wu = np.asarray(inputs["Wu"], dtype=np.float32)
    bu = np.asarray(inputs["bu"], dtype=np.float32)
    Wv = np.asarray(inputs["Wv"], dtype=np.float32)
    We = np.asarray(inputs["We"], dtype=np.float32)
    last_nodes = np.asarray(inputs["last_nodes"])

    maps = _prep_core_inputs(x, Wu, bu, Wv, We, last_nodes)
    if "nc" not in _CACHE:
        _CACHE["nc"] = _build_program()
    nc = _CACHE["nc"]
    res = run_bass_kernel_spmd(nc, maps, list(range(NCORES)), trace=TRACE)
    _CACHE["last_res"] = res
    outs = [np.asarray(r["out"], dtype=np.float32) for r in res.results]
    return np.concatenate(outs, axis=0)  # [B, O, D]


if __name__ == "__main__":
    rng = np.random.default_rng(0)
    x = rng.standard_normal((B, O, N, D), dtype=np.float32)
    Wu = rng.standard_normal((O, D, D), dtype=np.float32) * 0.09
    bu = np.zeros((O, D), np.float32)
    Wv = rng.standard_normal((O, D, D), dtype=np.float32) * 0.09
    We = rng.standard_normal((O, D), dtype=np.float32) * 0.09
    ln = rng.integers(0, N, size=(B, O)).astype(np.int64)
    out = kernel(x=x, Wu=Wu, bu=bu, Wv=Wv, We=We, last_nodes=ln)
    print(out.shape, out.dtype)


# revision 33
# speedup vs baseline: 1.1766x; 1.1766x over previous
"""AttnReadout kernel for Trainium2, 8 NeuronCores, data-parallel over batch.

Math (per batch b, head i):
  c[i,e]    = bu[i,e] + sum_d Wv[i,e,d] * x[b, i, last_nodes[b,i], d]
  z[t,e]    = sum_d x[b,t,d] * Wu[i,e,d]          (t over O*N = 8192 tokens)
  s[t,e]    = sigmoid(z[t,e] + c[i,e])
  score[t]  = sum_e We[i,e] * s[t,e]
  alpha     = softmax(score)        (scores bounded by |We|_1, so exp without
                                     max-subtraction is safe; softmax is
                                     shift-invariant so results match)
  out[b,i]  = sum_t alpha[t] * x[b,t,:]

Trick: sigmoid(v) = (1 + tanh(v/2))/2 and the We-dot is linear, so
  score = sum_e (We_e/2)*tanh((z_e + c_e)/2) + sum_e We_e/2
Using tanh keeps every ACT function (tanh, exp, identity) in the single
`exp_and_others` table set -> no ACT table reloads. The /2 factors are
folded into the uploaded weights (exact in bf16), the +sum(We)/2 into the
exp bias.

v2 pipeline (per core, 4 samples), driven by trace analysis of v1:
the kernel is a 2-stream race: the PE LDWEIGHTS stream (~1 col/cycle @
1.2 GHz; score-matmul stationaries dominate it) and the ACT stream
(1 elem/lane/cyc @ 1.2 GHz + ~352 cyc/call overhead). v1 lost ~35 us to
startup (12.5 us to first tanh), sample-boundary PE-queue clogs
(32-matmul wsum blocks), and tanh call-count overhead (64 calls/core).

v2:
  - z regions double-buffer across two 3-bank PSUM pools (1536 f32 each);
    6 tanh calls per head (48/core vs 64), A/B alternation lines up across
    heads so no pool-reuse stall.
  - per-sample scores/u/Z tile occupies one PSUM bank (scp, bufs=2); Z
    goes to the bank's padding bytes so nothing overlaps.
  - all small weights ship as one blob DMA; xt slices lead the DMA ring;
    warmup matmuls + the ch bias chain fill the HAM-cold window, so the
    first tanh fires at ~3.5 us instead of 12.5.
  - wsum (weighted-sum) matmuls spread 8-per-slot across the next
    sample's slots so the in-order PE queue never starves ACT.
"""

import numpy as np
import ml_dtypes

import concourse.bacc as bacc
import concourse.tile as tile
from concourse import mybir
from concourse.bass_utils import run_bass_kernel_spmd

BF = ml_dtypes.bfloat16
B, O, N, D = 32, 2, 4096, 128
NCORES = 8
BPC = B // NCORES          # samples per core
T = O * N                  # tokens per head (attention span is O*N)
NC64 = T // 128            # 64 token chunks of 128
WSB = 4                    # wsum chunks per emitted block

# z regions per head, balanced so no slot leaves the PE idle long enough
# to trip the HAM re-throttle; the first head starts with small regions so
# the ACT stream lights up as early as possible. Pools are picked by a
# global alternation counter. Pools hold 1536 f32 = 3 PSUM banks.
SIZES = [1024, 1536, 1536, 1536, 1536, 1024]
SIZES_FIRST = [512, 1024, 1536, 1536, 1536, 1536, 512]
ZCAP = 1536

# blob column layout (bf16): wuT [0:256], wvT [256:512], we2 [512:514],
# xlT [514:522], ones [522:523]
WU0, WV0, WE0, XL0, ON0, BLOBW = 0, 256, 512, 514, 522, 523


def _build_program():
    nc = bacc.Bacc("TRN2", target_bir_lowering=False)
    dt = mybir.dt
    f32, bf16 = dt.float32, dt.bfloat16

    xt_d = nc.dram_tensor("xt", [BPC, D, T], bf16, kind="ExternalInput")
    xn_d = nc.dram_tensor("xn", [BPC, 2, D, 32 * (D + 1)], bf16, kind="ExternalInput")
    wb_d = nc.dram_tensor("wb", [D, BLOBW], bf16, kind="ExternalInput")
    fb_d = nc.dram_tensor("fb", [D, 4], f32, kind="ExternalInput")  # bu|cw
    out_d = nc.dram_tensor("out", [BPC, O, D], f32, kind="ExternalOutput")

    Tanh = mybir.ActivationFunctionType.Tanh
    Exp = mybir.ActivationFunctionType.Exp
    Ident = mybir.ActivationFunctionType.Identity

    with tile.TileContext(nc) as tc:
        from contextlib import ExitStack

        with ExitStack() as ctx:
            singles = ctx.enter_context(tc.tile_pool(name="singles", bufs=1))
            xtp = ctx.enter_context(tc.tile_pool(name="xtp", bufs=3))
            xnp = ctx.enter_context(tc.tile_pool(name="xnp", bufs=3))
            za = ctx.enter_context(tc.tile_pool(name="za", bufs=1, space="PSUM"))
            zb = ctx.enter_context(tc.tile_pool(name="zb", bufs=1, space="PSUM"))
            scp = ctx.enter_context(tc.tile_pool(name="scp", bufs=2, space="PSUM"))
            sp = ctx.enter_context(tc.tile_pool(name="sp", bufs=4))
            smalls = ctx.enter_context(tc.tile_pool(name="smalls", bufs=2))

            wb_sb = singles.tile([D, BLOBW], bf16)
            nc.sync.dma_start(out=wb_sb, in_=wb_d[:])
            fb_sb = singles.tile([D, 4], f32)
            nc.sync.dma_start(out=fb_sb, in_=fb_d[:])

            wu = lambda i: wb_sb[:, WU0 + i * D : WU0 + (i + 1) * D]
            wv = lambda i: wb_sb[:, WV0 + i * D : WV0 + (i + 1) * D]
            we = lambda i: wb_sb[:, WE0 + i : WE0 + i + 1]
            ones_col = wb_sb[:, ON0 : ON0 + 1]

            samples = {}

            def preload(b):
                xt_sb = xtp.tile([D, T], bf16, tag="xt")
                bounds = (
                    [0, 512, 1536, 3072, 4608, 6144, 7680, T]
                    if b == 0
                    else [0, T // 2, T]
                )
                for lo, hi in zip(bounds[:-1], bounds[1:]):
                    nc.sync.dma_start(out=xt_sb[:, lo:hi], in_=xt_d[b, :, lo:hi])
                xn_sb = xnp.tile([D, NC64, D + 1], bf16, tag="xn")
                for g in range(2):
                    nc.sync.dma_start(
                        out=xn_sb[:, g * 32 : (g + 1) * 32, :],
                        in_=xn_d[b, g].rearrange("p (c d) -> p c d", c=32),
                    )
                # per-sample PSUM bank: scores [0:128] (viewed [i, c]),
                # u [128:257] on partitions 0-1 -- the xn ones column makes
                # u[:, 128] the softmax normalizer Z, for free.
                scu = scp.tile([D, 512], f32, tag="scu")
                scores = scu[:, 0:128].rearrange("p (i c) -> p i c", i=O)
                if b == BPC - 1:
                    # the last sample's weighted sum overlaps its own scoring;
                    # its u lives in the previous sample's (finished) bank so
                    # this sample's wedot start=True bank clears cannot wipe
                    # the accumulation bits mid-flight.
                    u_ap = samples[b - 1][5][0:2, 128:257]
                else:
                    u_ap = scu[0:2, 128:257]
                alpha_sb = smalls.tile([D, NC64, O], bf16, tag="alpha")
                samples[b] = (xt_sb, xn_sb, scores, u_ap, alpha_sb, scu)

            def emit_wedot(slot, b, i, col0, t_ap, size):
                scores = samples[b][2]
                for sub in range(size // D):
                    col = col0 + sub
                    nc.tensor.matmul(
                        scores[:, i, col : col + 1],
                        t_ap[:, sub * D : (sub + 1) * D],
                        we(i),
                        start=True,
                        stop=True,
                    )
                subs = size // D
                if b == BPC - 1 and i == O - 1:
                    # last sample, head 1: exp is chunk-local (raw exp, no
                    # max-subtraction), so score regions convert to alpha as
                    # they are produced and the weighted sum overlaps its own
                    # sample's scoring instead of draining serially at the end
                    exp_queue.append((slot, b, i, col0, subs))
                elif (col0 + subs) == NC64:
                    exp_queue.append((slot, b, i, 0, NC64))

            exp_queue = []    # (slot_pushed, b, i, col0, ncols)
            blocks_queued = [0] * BPC

            def emit_exp(b, i, col0, ncols):
                # runs >=1 slot after its wedot so the ACT FIFO never stalls
                # on the PE finishing the score columns
                scores, alpha_sb = samples[b][2], samples[b][4]
                nc.scalar.activation(
                    out=alpha_sb[:, col0 : col0 + ncols, i],
                    in_=scores[:, i, col0 : col0 + ncols],
                    func=Exp,
                    bias=fb_sb[:, 2 + i : 3 + i],
                )
                if i == O - 1:
                    ready = (col0 + ncols) // WSB
                    while blocks_queued[b] < ready:
                        deferred_wsum.append((b, blocks_queued[b]))
                        blocks_queued[b] += 1

            deferred_wsum = []  # (b, block_idx) queue

            def emit_wsum_block():
                b, blk = deferred_wsum.pop(0)
                _, xn_sb, _, u_ap, alpha_sb, _ = samples[b]
                for k in range(WSB):
                    c = blk * WSB + k
                    nc.tensor.matmul(
                        u_ap,
                        alpha_sb[:, c, :],
                        xn_sb[:, c, :],
                        start=(c == 0),
                        stop=(c == NC64 - 1),
                    )
                if blk == NC64 // WSB - 1:
                    zinv_sb = smalls.tile([O, 1], f32, tag="zinv")
                    nc.vector.reciprocal(out=zinv_sb, in_=u_ap[:, 128:129])
                    o_sb = smalls.tile([O, D], f32, tag="osb")
                    nc.vector.tensor_scalar_mul(o_sb, u_ap[:, 0:128], zinv_sb)
                    nc.sync.dma_start(out=out_d[b], in_=o_sb)

            preload(0)

            # startup: warmup matmuls heat the PE HAM clock gate while the
            # first xt slices stream in. The ch bias chain (c_ps in sample
            # 0's scu spare columns) interleaves so proj r0 is not gated on
            # a z-pool tile.
            zwarm = za.tile([D, ZCAP], f32, tag="z")
            scu0 = samples[0][5]
            for w in range(4):
                nc.tensor.matmul(zwarm[:, 0:D], wu(0), wv(1), start=True, stop=True)
            # per-(sample, head) tanh bias ch[e, j] = (xv + bu)/2, j = i*BPC + b
            # (wv and bu are uploaded pre-halved)
            for i in range(O):
                nc.tensor.matmul(
                    scu0[:, 264 + i * BPC : 264 + (i + 1) * BPC],
                    wv(i),
                    wb_sb[:, XL0 + i * BPC : XL0 + (i + 1) * BPC],
                    start=True,
                    stop=True,
                )
            ch_sb = singles.tile([D, O * BPC], f32)
            for i in range(O):
                nc.scalar.activation(
                    out=ch_sb[:, i * BPC : (i + 1) * BPC],
                    in_=scu0[:, 264 + i * BPC : 264 + (i + 1) * BPC],
                    func=Ident,
                    bias=fb_sb[:, i : i + 1],
                )
            for w in range(2):
                nc.tensor.matmul(zwarm[:, 0:D], wu(0), wv(1), start=True, stop=True)

            pending = None
            grc = 0  # global region counter: picks the A/B pool alternately
            slot = 0
            for b in range(BPC):
                for i in range(O):
                    off = 0
                    sizes = SIZES_FIRST if (b, i) == (0, 0) else SIZES
                    for r, size in enumerate(sizes):
                        if i == 0 and r == 0 and b + 1 < BPC:
                            preload(b + 1)
                        z = (za if grc % 2 == 0 else zb).tile([D, ZCAP], f32, tag="z")
                        grc += 1
                        xt_sb = samples[b][0]
                        for p0 in range(0, size, 512):
                            p1 = min(p0 + 512, size)
                            nc.tensor.matmul(
                                z[:, p0:p1],
                                wu(i),
                                xt_sb[:, off + p0 : off + p1],
                                start=True,
                                stop=True,
                            )
                        if pending is not None:
                            emit_wedot(slot, *pending)
                        if deferred_wsum:
                            emit_wsum_block()
                        t_sb = sp.tile([D, ZCAP], bf16, tag="t")
                        j = i * BPC + b
                        nc.scalar.activation(
                            out=t_sb[:, :size],
                            in_=z[:, :size],
                            func=Tanh,
                            bias=ch_sb[:, j : j + 1],
                        )
                        while exp_queue and exp_queue[0][0] < slot - 1:
                            _, eb, ei, ecol0, encols = exp_queue.pop(0)
                            emit_exp(eb, ei, ecol0, encols)
                        pending = (b, i, off // D, t_sb[:, :size], size)
                        off += size
                        slot += 1
            emit_wedot(slot, *pending)
            while exp_queue:
                _, eb, ei, ecol0, encols = exp_queue.pop(0)
                emit_exp(eb, ei, ecol0, encols)
            while deferred_wsum:
                emit_wsum_block()

    nc.compile()
    return nc


def _prep_core_inputs(x, Wu, bu, Wv, We, last_nodes):
    """Host-side input marshalling: dtype cast + layout (weights pre-halved
    for the tanh formulation). Returns per-core input maps."""
    x = np.ascontiguousarray(x, dtype=np.float32)
    ln = np.asarray(last_nodes).astype(np.int64)
    xb = x.reshape(B, T, D)
    xbf = xb.astype(BF)                                  # [B, T, D] bf16
    xt = np.ascontiguousarray(xbf.transpose(0, 2, 1))    # [B, D, T]
    # natural-chunked layout with a trailing ones column per chunk (so the
    # weighted sum also produces the softmax normalizer Z):
    # xn[b, g, p, cc*(D+1) + d] = xb[b, (g*32 + cc)*128 + p, d]; d=D slot = 1
    xn5 = xbf.reshape(B, 2, 32, D, D).transpose(0, 1, 3, 2, 4)   # [b,g,p,cc,d]
    xn = np.empty((B, 2, D, 32, D + 1), BF)
    xn[..., :D] = xn5
    xn[..., D] = BF(1.0)
    xn = np.ascontiguousarray(xn.reshape(B, 2, D, 32 * (D + 1)))
    # x_last gather, transposed: xlT[d, j], j = i*BPC + b_local
    xl = xb[np.arange(B)[:, None], ln + np.arange(O)[None, :] * N]   # [B, O, D] f32
    # wuT[d, i, e] = Wu[i, e, d] / 2  (tanh halving, exact in bf16)
    wuT = (Wu * 0.5).transpose(2, 0, 1).reshape(D, O * D).astype(BF)
    wvT = (Wv * 0.5).transpose(2, 0, 1).reshape(D, O * D).astype(BF)
    we2 = (We * 0.5).astype(BF).T                        # [e, i] = We[i, e]/2
    bu2 = np.ascontiguousarray((bu * 0.5).astype(np.float32).T)  # [e, i]
    # exp bias: cw[i] = sum_e We[i, e]/2, replicated on all partitions
    cw = np.float32(0.5) * We.astype(np.float32).sum(axis=1)     # [O]
    cw2 = np.broadcast_to(cw[None, :], (D, O)).astype(np.float32)
    fb = np.ascontiguousarray(np.concatenate([bu2, cw2], axis=1))  # [D, 4]
    ones = np.ones((D, 1), BF)

    maps = []
    for core in range(NCORES):
        sl = slice(core * BPC, (core + 1) * BPC)
        xlc = xl[sl]                                     # [BPC, O, D]
        xlT = xlc.transpose(2, 1, 0).reshape(D, O * BPC).astype(BF)
        wb = np.ascontiguousarray(
            np.concatenate([wuT, wvT, we2, xlT, ones], axis=1)
        )                                                # [D, 523] bf16
        maps.append({"xt": xt[sl], "xn": xn[sl], "wb": wb, "fb": fb})
    return maps


_CACHE = {}
TRACE = False


def kernel(**inputs):
    x = np.asarray(inputs["x"])
    Wu = np.asarray(inputs["Wu"], dtype=np.float32)
    bu = np.asarray(inputs["bu"], dtype=np.float32)
    Wv = np.asarray(inputs["Wv"], dtype=np.float32)
    We = np.asarray(inputs["We"], dtype=np.float32)
    last_nodes = np.asarray(inputs["last_nodes"])

    maps = _prep_core_inputs(x, Wu, bu, Wv, We, last_nodes)
    if "nc" not in _CACHE:
        _CACHE["nc"] = _build_program()
    nc = _CACHE["nc"]
    res = run_bass_kernel_spmd(nc, maps, list(range(NCORES)), trace=TRACE)
    _CACHE["last_res"] = res
    outs = [np.asarray(r["out"], dtype=np.float32) for r in res.results]
    return np.concatenate(outs, axis=0)  # [B, O, D]


if __name__ == "__main__":
    rng = np.random.default_rng(0)
    x = rng.standard_normal((B, O, N, D), dtype=np.float32)
    Wu = rng.standard_normal((O, D, D), dtype=np.float32) * 0.09
    bu = np.zeros((O, D), np.float32)
    Wv = rng.standard_normal((O, D, D), dtype=np.float32) * 0.09
    We = rng.standard_normal((O, D), dtype=np.float32) * 0.09
    ln = rng.integers(0, N, size=(B, O)).astype(np.int64)
    out = kernel(x=x, Wu=Wu, bu=bu, Wv=Wv, We=We, last_nodes=ln)
    print(out.shape, out.dtype)
